# revision 59
# baseline (speedup 1.0000x reference)
"""Trainium2 Bass kernel for multi-head attention (B=2, S=2048, D=2048, 16 heads).

Sharding: 8 cores = 2 batch groups (data parallel) x 4 tensor-parallel ranks.
Each core computes QKV + attention for its 4 heads over its batch element.
Per 512-row query chunk the cores exchange their (normalized) attention
outputs O^T with an 8-way AllToAll (one per head-pair half), then each core
contracts the full 2048-dim O rows of the query subtile it owns against the
full wo^T.  The A2A must span all 8 cores (mesh needs >4), so each core
writes its blocks into both batch-groups' destination slots and picks the
correct source half with rank-conditional DMAs (cc_rank).

Layout:
- All device matmuls contract over the partition dim.  Host pre-transposes:
  xt = x^T, per-head q/k weights as [d, hd] blocks, wv as [d, vcols],
  woT = wo^T.
- Q/K are produced in [hd, s] layout (RoPE pairs permuted even|odd so the
  rotation acts on partition halves); V is produced directly in natural
  [s, hd] layout (stationary = xt tile), so no PE transposes anywhere.
- Scores are computed transposed [k, q]:  exp tiles feed PV directly
  (O^T accumulates in PSUM) and the softmax denominator comes from a
  ones-vector matmul; normalization multiplies O^T by a partition-broadcast
  reciprocal.  Softmax scale is folded into the Exp activation.

Scheduling:
- QKV runs s-chunk-major so the first matmul starts after ~2.5 MB of DMA.
- Chunk 0's attention runs as soon as s-chunk 0 is done, so its AllToAll
  fires ~110us in and absorbs core launch skew under phase-A compute.
- Chunk 0's out-projection matmuls are deferred to the very end (with the
  last chunk's hp0 half preloaded), hiding the final AllToAll latency.
  DVE "shield" copies detach those matmuls from the conservative
  dynamic-DMA/collective semaphore chains.
"""

import sys
import numpy as np
import ml_dtypes

sys.path.insert(0, "/opt/trn_rl_repo")

B, S, D = 2, 2048, 2048
NH, HD = 16, 128
TP = 4            # tensor-parallel ranks per batch group
HL = NH // TP     # heads per core = 4
NDT = D // 128    # 16 d-tiles
NSC = 4           # 512-col s chunks
NQT = S // 128    # 16
NQC = 4           # 512-row query chunks
SM_SCALE = float(HD) ** -0.5
GROUPS8 = [[0, 1, 2, 3, 4, 5, 6, 7]]
CHUNK_ORDER = [0, 2, 3, 1]

_cache = {}


def _build_graph():
    import concourse.mybir as mybir
    import concourse.tile as tile
    from concourse import bacc
    from concourse.bass_isa import ReduceOp

    f32 = mybir.dt.float32
    bf16 = mybir.dt.bfloat16
    AF = mybir.ActivationFunctionType
    OP = mybir.AluOpType

    nc = bacc.Bacc("TRN2", target_bir_lowering=False, debug=False, num_devices=8)

    xt_ext = nc.declare_dram_parameter("xt", [D, S], bf16, isOutput=False)
    wqk_ext = nc.declare_dram_parameter("wqk", [2 * HL * 128, NDT * 128], bf16,
                                        isOutput=False)
    wv_ext = nc.declare_dram_parameter("wv", [D, HL * HD], bf16, isOutput=False)
    c2_ext = nc.declare_dram_parameter("c2", [128, S], f32, isOutput=False)
    s2_ext = nc.declare_dram_parameter("s2", [128, S], f32, isOutput=False)
    maskT_ext = nc.declare_dram_parameter("maskT", [128, 128], bf16, isOutput=False)
    woT_ext = nc.declare_dram_parameter("woT", [D, D], bf16, isOutput=False)
    out_ext = nc.declare_dram_parameter("out", [NQC * 128, D], bf16, isOutput=True)

    with tile.TileContext(nc) as tc:
        with tc.tile_pool(name="pers", bufs=1) as pers, \
             tc.tile_pool(name="dram", bufs=1, space="DRAM") as dram, \
             tc.tile_pool(name="att", bufs=4) as attp, \
             tc.tile_pool(name="psS", bufs=3, space="PSUM") as psS, \
             tc.tile_pool(name="psPV", bufs=3, space="PSUM") as psPV:
            qk_bf = [pers.tile([128, S], bf16, tag=f"qk{i}", name=f"qk{i}")
                     for i in range(2 * HL)]            # 0-3 q heads, 4-7 k heads
            v_bf = [pers.tile([128, HL * HD], bf16, tag=f"v{t}", name=f"v{t}")
                    for t in range(NQT)]                # [s-tile, 4*hd]
            mtri = pers.tile([128, 128], bf16, tag="mtri", name="mtri")
            # pre-zeroed exp tiles for narrowed diagonal score tiles:
            # activation only writes cols [j*128:512], the strip stays zero
            e_diag = {}
            for j in range(1, 4):
                for i in range(2):
                    t = pers.tile([128, 512], bf16, tag=f"ed{j}{i}",
                                  name=f"ed{j}{i}")
                    nc.vector.memset(t[:, 0:j * 128], 0.0)
                    e_diag[(j, i)] = t
            rank = nc.sync.cc_rank(replica_groups=GROUPS8)
            is_b0 = rank < 4
            is_b1 = rank >= 4
            a2a_out = {}

            def attention(qc, hp):
                qcl = slice(qc * 512, (qc + 1) * 512)
                nkt = qc * 4 + 4
                o2p = attp.tile([128, 1024], bf16, tag="o2p",
                                name=f"o2p_{qc}_{hp}", bufs=2)
                ps_pv = [psPV.tile([128, 512], f32, tag="pv",
                                   name=f"pv_{qc}_{hp}_{i}") for i in range(2)]
                dacc = [attp.tile([128, 512], f32, tag=f"dacc{i}",
                                  name=f"dacc_{qc}_{hp}_{i}", bufs=1)
                        for i in range(2)]
                for kt in range(nkt):
                    # stage-ordered so the DVE queue never interleaves a
                    # mask add behind an exp-dependent accumulator add.
                    # Diagonal tile j: cols < j*128 are fully masked, so
                    # scores/exp/mask/acc all narrow to [lo:512] and the PV
                    # matmul reads a pre-zeroed strip.
                    j = kt - qc * 4
                    lo = j * 128 if j >= 1 else 0
                    ps_l, e_l = [], []
                    for i in range(2):
                        h = 2 * hp + i
                        ps_s = psS.tile([128, 512], f32, tag="s",
                                        name=f"s_{qc}_{hp}_{kt}_{i}")
                        nc.tensor.matmul(
                            ps_s[:, lo:512],
                            qk_bf[HL + h][:, kt * 128:(kt + 1) * 128],
                            qk_bf[h][:, qc * 512 + lo:(qc + 1) * 512],
                            start=True, stop=True)
                        ps_l.append(ps_s)
                    if j >= 0:
                        for i in range(2):
                            nc.vector.tensor_tensor(
                                out=ps_l[i][:, j * 128:(j + 1) * 128],
                                in0=ps_l[i][:, j * 128:(j + 1) * 128],
                                in1=mtri[:], op=OP.add)
                    for i in range(2):
                        if j >= 1:
                            e_sb = e_diag[(j, i)]
                        else:
                            e_sb = attp.tile([128, 512], bf16, tag="e",
                                             name=f"e_{qc}_{hp}_{kt}_{i}",
                                             bufs=4)
                        nc.scalar.activation(e_sb[:, lo:512],
                                             ps_l[i][:, lo:512], AF.Exp,
                                             scale=SM_SCALE)
                        e_l.append(e_sb)
                    for i in range(2):
                        if kt == 0:
                            nc.vector.tensor_copy(dacc[i][:], e_l[i][:])
                        else:
                            nc.vector.tensor_tensor(
                                out=dacc[i][:, lo:512],
                                in0=dacc[i][:, lo:512],
                                in1=e_l[i][:, lo:512], op=OP.add)
                    for i in range(2):
                        h = 2 * hp + i
                        # kt=0 is always full-width (j<=0), so the partial
                        # accumulates land on an initialized full bank
                        nc.tensor.matmul(
                            ps_pv[i][:, lo:512],
                            v_bf[kt][:, h * 128:(h + 1) * 128],
                            e_l[i][:, lo:512],
                            start=(kt == 0), stop=(kt == nkt - 1))
                for i in range(2):
                    nc.gpsimd.partition_all_reduce(dacc[i][:], dacc[i][:],
                                                   128, ReduceOp.add)
                for i in range(2):
                    rb = attp.tile([128, 512], f32, tag="rb",
                                   name=f"rb_{qc}_{hp}_{i}", bufs=1)
                    nc.vector.reciprocal_approx_fast(rb[:], dacc[i][:])
                    # o2p column layout is g-major (g = r*2 + i: dest rank r,
                    # pair member i) so the staging DMA is 3-dim
                    nc.vector.tensor_tensor(
                        out=o2p[:].rearrange("p (r i f) -> p i r f",
                                             i=2, f=128)[:, i],
                        in0=ps_pv[i][:].rearrange("p (r f) -> p r f", f=128),
                        in1=rb[:].rearrange("p (r f) -> p r f", f=128),
                        op=OP.mult)
                # stage O^T blocks to DRAM grouped by destination core
                # (same block for both batch-groups' slots) and exchange
                a_in = dram.tile([2048, 128], bf16, tag=f"ain_{qc}_{hp}",
                                 name=f"ain_{qc}_{hp}")
                src = o2p[:].rearrange("p (g f) -> p g f", f=128)
                # only the half read by this core's batch group needs real
                # data; the other half is junk the receivers never read
                for half, cond in ((0, is_b0), (1, is_b1)):
                    dst = a_in[half * 1024:(half + 1) * 1024, :].rearrange(
                        "(g p) f -> p g f", p=128)
                    nc.sync.dma_start(out=dst, in_=src, cond=cond)
                a_out = dram.tile([2048, 128], bf16, tag=f"aout_{qc}_{hp}",
                                  name=f"aout_{qc}_{hp}")
                nc.gpsimd.collective_compute(
                    "AllToAll", OP.bypass, replica_groups=GROUPS8,
                    ins=[a_in[:].opt()], outs=[a_out[:].opt()])
                a2a_out[(qc, hp)] = a_out

            # ---------------- Phase A: QKV projection + RoPE ----------------
            # (+ chunk 0's attention as soon as s-chunk 0 is complete)
            with tc.tile_pool(name="pha", bufs=1) as pha, \
                 tc.tile_pool(name="rope", bufs=1) as ropep, \
                 tc.tile_pool(name="psA", bufs=2, space="PSUM") as psA:
                wq_sb = [pha.tile([128, NDT * 128], bf16, tag=f"wq{et}",
                                  name=f"wq{et}") for et in range(2 * HL)]
                wv_sb = [pha.tile([128, HL * HD], bf16, tag=f"wv{dt}",
                                  name=f"wv{dt}") for dt in range(NDT)]
                xt_t = {}
                tabs = {}

                def prefetch_xt(sc):
                    for dt in range(NDT):
                        t = pha.tile([128, 512], bf16, tag=f"xt{dt}",
                                     name=f"xt_{dt}_{sc}", bufs=2)
                        nc.sync.dma_start(
                            out=t[:],
                            in_=xt_ext[dt * 128:(dt + 1) * 128,
                                       sc * 512:(sc + 1) * 512])
                        xt_t[(dt, sc)] = t

                def prefetch_tab(sc):
                    cl = slice(sc * 512, (sc + 1) * 512)
                    ct = pha.tile([128, 512], f32, tag="c2",
                                  name=f"c2_{sc}", bufs=2)
                    st_ = pha.tile([128, 512], f32, tag="s2",
                                   name=f"s2_{sc}", bufs=2)
                    nc.sync.dma_start(out=ct[:], in_=c2_ext[:, cl])
                    nc.sync.dma_start(out=st_[:], in_=s2_ext[:, cl])
                    tabs[sc] = (ct, st_)

                # DMA issue order tuned so compute starts after ~2.5 MB and
                # the rotary tables are resident before the first RoPE drain
                nc.sync.dma_start(out=wq_sb[0][:], in_=wqk_ext[0:128, :])
                prefetch_tab(0)
                prefetch_xt(0)
                for et in range(1, 2 * HL):
                    nc.sync.dma_start(out=wq_sb[et][:],
                                      in_=wqk_ext[et * 128:(et + 1) * 128, :])
                for dt in range(NDT):
                    nc.sync.dma_start(out=wv_sb[dt][:],
                                      in_=wv_ext[dt * 128:(dt + 1) * 128, :])
                nc.sync.dma_start(out=mtri[:], in_=maskT_ext[:])
                prefetch_xt(1)
                prefetch_tab(1)

                for sc in range(NSC):
                    ct, st_ = tabs[sc]
                    for et in range(2 * HL):
                        ps = psA.tile([128, 512], f32, tag="psA",
                                      name=f"psA_{sc}_{et}")
                        for dt in range(NDT):
                            nc.tensor.matmul(
                                ps[:], wq_sb[et][:, dt * 128:(dt + 1) * 128],
                                xt_t[(dt, sc)][:],
                                start=(dt == 0), stop=(dt == NDT - 1))
                        # u = [r*c; i*c]; w = [-i*s; r*s] (s2 = [-sin; sin],
                        # cross-partition reads stay on the PSUM operand);
                        # qk = u + w = [r*c - i*s; i*c + r*s]
                        cl = slice(sc * 512, (sc + 1) * 512)
                        u = ropep.tile([128, 512], f32, tag="t1",
                                       name=f"t1_{sc}_{et}")
                        w = ropep.tile([128, 512], f32, tag="t2",
                                       name=f"t2_{sc}_{et}")
                        nc.vector.tensor_tensor(out=u[:], in0=ps[:],
                                                in1=ct[:], op=OP.mult)
                        nc.vector.tensor_tensor(out=w[0:64, :],
                                                in0=ps[64:128, :],
                                                in1=st_[0:64, :],
                                                op=OP.mult)
                        nc.vector.tensor_tensor(out=w[64:128, :],
                                                in0=ps[0:64, :],
                                                in1=st_[64:128, :],
                                                op=OP.mult)
                        nc.vector.tensor_tensor(out=qk_bf[et][:, cl],
                                                in0=u[:], in1=w[:], op=OP.add)
                    for stl in range(4):
                        st = sc * 4 + stl
                        psv = psA.tile([128, 512], f32, tag="psA",
                                       name=f"psV_{st}")
                        for dt in range(NDT):
                            nc.tensor.matmul(
                                psv[:],
                                xt_t[(dt, sc)][:, stl * 128:(stl + 1) * 128],
                                wv_sb[dt][:],
                                start=(dt == 0), stop=(dt == NDT - 1))
                        nc.scalar.copy(v_bf[st][:], psv[:])
                    if sc + 2 < NSC:
                        prefetch_xt(sc + 2)
                        prefetch_tab(sc + 2)
                    if sc == 0:
                        attention(0, 0)
                        attention(0, 1)

            # -------- Phase B: remaining attention + A2A + out-proj --------
            with tc.tile_pool(name="phb", bufs=1) as phb, \
                 tc.tile_pool(name="psPR", bufs=2, space="PSUM") as psPR:
                woT_sb = [phb.tile([128, D], bf16, tag=f"wo{k}", name=f"wo{k}")
                          for k in range(NDT)]
                for k in range(NDT):
                    nc.sync.dma_start(out=woT_sb[k][:],
                                      in_=woT_ext[k * 128:(k + 1) * 128, :])

                def load_lhs_hp(qc, hp):
                    # lhs_hp block g = r*2 + i holds global ocol block (head)
                    # k = r*4 + 2*hp + i.  Separate tiles per hp so the
                    # conservative whole-tile deps of the conditional DMAs
                    # don't make the hp0 matmuls wait on the hp1 AllToAll.
                    t = attp.tile([128, 1024], bf16, tag=f"lhs{hp}",
                                  name=f"lhs_{qc}_{hp}", bufs=2)
                    a_out = a2a_out[(qc, hp)]
                    dst = t[:].rearrange("p (g f) -> p g f", f=128)
                    for b, cond in ((0, is_b0), (1, is_b1)):
                        src = a_out[b * 1024:(b + 1) * 1024, :].rearrange(
                            "(g p) f -> p g f", p=128)
                        nc.sync.dma_start(out=dst, in_=src, cond=cond)
                    return t

                def shield(t, tag, bufs=1):
                    # re-copy on the DVE so later consumers depend on the
                    # copy, not on the dynamic-DMA/collective semaphore
                    # chain (which conservatively waits for ALL later
                    # collectives)
                    c = attp.tile([128, 1024], bf16, tag=tag,
                                  name=f"sh_{tag}", bufs=bufs)
                    nc.vector.tensor_copy(c[:], t[:])
                    return c

                def outproj(qc, lhs=None):
                    if lhs is None:
                        lhs = [None, None]
                    lhs = [lhs[hp] if lhs[hp] is not None
                           else load_lhs_hp(qc, hp) for hp in range(2)]
                    # ec columns run in pairs, hp0 blocks first across the
                    # pair, so the first half of the contraction can run
                    # while the hp1 AllToAll is still in flight
                    for eca, ecb in ((0, 1), (2, 3)):
                        pss = {ec: psPR.tile([128, 512], f32, tag="pr",
                                             name=f"pr_{qc}_{ec}")
                               for ec in (eca, ecb)}
                        for hp in range(2):
                            for ec in (eca, ecb):
                                for n in range(8):
                                    r, i = divmod(n, 2)
                                    k = r * HL + 2 * hp + i
                                    nc.tensor.matmul(
                                        pss[ec][:],
                                        lhs[hp][:, n * 128:(n + 1) * 128],
                                        woT_sb[k][:, ec * 512:(ec + 1) * 512],
                                        start=(hp == 0 and n == 0),
                                        stop=(hp == 1 and n == 7))
                        for ec in (eca, ecb):
                            fin = attp.tile([128, 512], bf16, tag="fin",
                                            name=f"fin_{qc}_{ec}", bufs=2)
                            nc.scalar.copy(fin[:], pss[ec][:])
                            nc.sync.dma_start(
                                out=out_ext[qc * 128:(qc + 1) * 128,
                                            ec * 512:(ec + 1) * 512],
                                in_=fin[:])

                # the out-projections of chunks 0 and 3 are deferred (with
                # shielded lhs tiles) to the very end, so ~45us of matmuls
                # covers the final AllToAll; only the last chunk's hp1 half
                # remains exposed
                q0, qa, qb, qlast = CHUNK_ORDER
                attention(qa, 0)
                raw0 = [load_lhs_hp(q0, hp) for hp in range(2)]
                attention(qa, 1)
                attention(qb, 0)
                outproj(qa)
                attention(qb, 1)
                lhs0 = [shield(raw0[hp], f"lhsc{hp}") for hp in range(2)]
                lhsb = [shield(load_lhs_hp(qb, hp), f"lhs{hp}", bufs=2)
                        for hp in range(2)]
                attention(qlast, 0)
                lhs_last_h0 = shield(load_lhs_hp(qlast, 0), "lhsd0")
                attention(qlast, 1)
                outproj(qb, lhs=lhsb)
                outproj(q0, lhs=lhs0)
                outproj(qlast, lhs=[lhs_last_h0, None])
    nc.finalize()
    return nc


def _prep_inputs(x, freqs_cos, freqs_sin, mask, wqkv, wo):
    bf = ml_dtypes.bfloat16
    perm = np.concatenate([np.arange(0, HD, 2), np.arange(1, HD, 2)])
    mask2d = np.asarray(mask, np.float32).reshape(S, S)
    maskT = np.ascontiguousarray(
        np.maximum(mask2d[0:128, 0:128].T, -1e30)).astype(ml_dtypes.bfloat16)
    cosT = np.asarray(freqs_cos, np.float32).T   # [64, S]
    sinT = np.asarray(freqs_sin, np.float32).T
    c2 = np.ascontiguousarray(np.concatenate([cosT, cosT], axis=0))
    s2 = np.ascontiguousarray(np.concatenate([-sinT, sinT], axis=0))
    wqkv = np.asarray(wqkv, np.float32)
    wo = np.asarray(wo, np.float32)
    x = np.asarray(x, np.float32)
    woT = np.ascontiguousarray(wo.T).astype(bf)   # [2048 o, 2048 e]

    in_maps = []
    for c in range(8):
        b, r = divmod(c, TP)
        heads = list(range(r * HL, (r + 1) * HL))
        # q/k weights: per (sec, head) block in SBUF layout [128 p=d%128,
        # (dt c)=hd], i.e. transpose of blk[c, dt*128+p]
        rows = []
        for sec in range(2):
            for h in heads:
                blk = wqkv[sec * D + h * HD: sec * D + (h + 1) * HD][perm]
                b3 = blk.reshape(HD, NDT, 128)          # [hd, dt, p]
                rows.append(np.transpose(b3, (2, 1, 0)).reshape(128, -1))
        wqk = np.ascontiguousarray(np.concatenate(rows, axis=0)).astype(bf)
        wv = np.ascontiguousarray(np.concatenate(
            [wqkv[2 * D + h * HD: 2 * D + (h + 1) * HD].T for h in heads],
            axis=1)).astype(bf)                          # [2048, 512]
        xt = np.ascontiguousarray(x[b].T).astype(bf)
        in_maps.append({"xt": xt, "wqk": wqk, "wv": wv, "c2": c2, "s2": s2,
                        "maskT": maskT, "woT": woT})
    return in_maps


def kernel(x, freqs_cos, freqs_sin, mask, wqkv, wo, input_pos=None,
           _want_res=False, _trace=False, _tmpdir=None):
    from concourse.bass_utils import run_bass_kernel_spmd

    if "nc" not in _cache:
        _cache["nc"] = _build_graph()
    nc = _cache["nc"]

    in_maps = _prep_inputs(x, freqs_cos, freqs_sin, mask, wqkv, wo)
    kw = {}
    if _trace:
        kw = dict(trace=True, tmpdir=_tmpdir)
    res = run_bass_kernel_spmd(nc, in_maps, list(range(8)), **kw)

    y = np.empty((B, S, D), np.float32)
    for c in range(8):
        b, r = divmod(c, TP)
        oc = np.asarray(res.results[c]["out"], np.float32)
        for qc in range(NQC):
            qt = 4 * qc + r
            y[b, qt * 128:(qt + 1) * 128, :] = oc[qc * 128:(qc + 1) * 128]
    if _want_res:
        return y, res
    return y


# revision 60
# speedup vs baseline: 1.0027x; 1.0027x over previous
"""Trainium2 Bass kernel for multi-head attention (B=2, S=2048, D=2048, 16 heads).

Sharding: 8 cores = 2 batch groups (data parallel) x 4 tensor-parallel ranks.
Each core computes QKV + attention for its 4 heads over its batch element.
Per 512-row query chunk the cores exchange their (normalized) attention
outputs O^T with an 8-way AllToAll (one per head-pair half), then each core
contracts the full 2048-dim O rows of the query subtile it owns against the
full wo^T.  The A2A must span all 8 cores (mesh needs >4), so each core
writes its blocks into both batch-groups' destination slots and picks the
correct source half with rank-conditional DMAs (cc_rank).

Layout:
- All device matmuls contract over the partition dim.  Host pre-transposes:
  xt = x^T, per-head q/k weights as [d, hd] blocks, wv as [d, vcols],
  woT = wo^T.
- Q/K are produced in [hd, s] layout (RoPE pairs permuted even|odd so the
  rotation acts on partition halves); V is produced directly in natural
  [s, hd] layout (stationary = xt tile), so no PE transposes anywhere.
- Scores are computed transposed [k, q]:  exp tiles feed PV directly
  (O^T accumulates in PSUM) and the softmax denominator comes from a
  ones-vector matmul; normalization multiplies O^T by a partition-broadcast
  reciprocal.  Softmax scale is folded into the Exp activation.

Scheduling:
- QKV runs s-chunk-major so the first matmul starts after ~2.5 MB of DMA.
- Chunk 0's attention runs as soon as s-chunk 0 is done, so its AllToAll
  fires ~110us in and absorbs core launch skew under phase-A compute.
- Chunk 0's out-projection matmuls are deferred to the very end (with the
  last chunk's hp0 half preloaded), hiding the final AllToAll latency.
  DVE "shield" copies detach those matmuls from the conservative
  dynamic-DMA/collective semaphore chains.
"""

import sys
import numpy as np
import ml_dtypes

sys.path.insert(0, "/opt/trn_rl_repo")

B, S, D = 2, 2048, 2048
NH, HD = 16, 128
TP = 4            # tensor-parallel ranks per batch group
HL = NH // TP     # heads per core = 4
NDT = D // 128    # 16 d-tiles
NSC = 4           # 512-col s chunks
NQT = S // 128    # 16
NQC = 4           # 512-row query chunks
SM_SCALE = float(HD) ** -0.5
GROUPS8 = [[0, 1, 2, 3, 4, 5, 6, 7]]
CHUNK_ORDER = [0, 2, 3, 1]

_cache = {}


def _build_graph():
    import concourse.mybir as mybir
    import concourse.tile as tile
    from concourse import bacc
    from concourse.bass_isa import ReduceOp

    f32 = mybir.dt.float32
    bf16 = mybir.dt.bfloat16
    AF = mybir.ActivationFunctionType
    OP = mybir.AluOpType

    nc = bacc.Bacc("TRN2", target_bir_lowering=False, debug=False, num_devices=8)

    xt_ext = nc.declare_dram_parameter("xt", [D, S], bf16, isOutput=False)
    wqk_ext = nc.declare_dram_parameter("wqk", [2 * HL * 128, NDT * 128], bf16,
                                        isOutput=False)
    wv_ext = nc.declare_dram_parameter("wv", [D, HL * HD], bf16, isOutput=False)
    c2_ext = nc.declare_dram_parameter("c2", [128, S], f32, isOutput=False)
    s2_ext = nc.declare_dram_parameter("s2", [128, S], f32, isOutput=False)
    maskT_ext = nc.declare_dram_parameter("maskT", [128, 128], bf16, isOutput=False)
    woT_ext = nc.declare_dram_parameter("woT", [D, D], bf16, isOutput=False)
    out_ext = nc.declare_dram_parameter("out", [NQC * 128, D], bf16, isOutput=True)

    with tile.TileContext(nc) as tc:
        with tc.tile_pool(name="pers", bufs=1) as pers, \
             tc.tile_pool(name="dram", bufs=1, space="DRAM") as dram, \
             tc.tile_pool(name="att", bufs=4) as attp, \
             tc.tile_pool(name="psS", bufs=3, space="PSUM") as psS, \
             tc.tile_pool(name="psPV", bufs=3, space="PSUM") as psPV:
            qk_bf = [pers.tile([128, S], bf16, tag=f"qk{i}", name=f"qk{i}")
                     for i in range(2 * HL)]            # 0-3 q heads, 4-7 k heads
            v_bf = [pers.tile([128, HL * HD], bf16, tag=f"v{t}", name=f"v{t}")
                    for t in range(NQT)]                # [s-tile, 4*hd]
            mtri = pers.tile([128, 128], bf16, tag="mtri", name="mtri")
            # pre-zeroed exp tiles for narrowed diagonal score tiles:
            # activation only writes cols [j*128:512], the strip stays zero
            e_diag = {}
            for j in range(1, 4):
                for i in range(2):
                    t = pers.tile([128, 512], bf16, tag=f"ed{j}{i}",
                                  name=f"ed{j}{i}")
                    nc.vector.memset(t[:, 0:j * 128], 0.0)
                    e_diag[(j, i)] = t
            rank = nc.sync.cc_rank(replica_groups=GROUPS8)
            is_b0 = rank < 4
            is_b1 = rank >= 4
            a2a_out = {}

            # tiny dummy AllToAll fired immediately: absorbs core launch
            # skew while input DMAs stream (nothing depends on its output),
            # so the real collectives later see aligned peers
            warm_in = dram.tile([8, 128], bf16, tag="warm_i", name="warm_i")
            warm_out = dram.tile([8, 128], bf16, tag="warm_o", name="warm_o")
            nc.gpsimd.collective_compute(
                "AllToAll", OP.bypass, replica_groups=GROUPS8,
                ins=[warm_in[:].opt()], outs=[warm_out[:].opt()])

            def attention(qc, hp):
                qcl = slice(qc * 512, (qc + 1) * 512)
                nkt = qc * 4 + 4
                o2p = attp.tile([128, 1024], bf16, tag="o2p",
                                name=f"o2p_{qc}_{hp}", bufs=2)
                ps_pv = [psPV.tile([128, 512], f32, tag="pv",
                                   name=f"pv_{qc}_{hp}_{i}") for i in range(2)]
                dacc = [attp.tile([128, 512], f32, tag=f"dacc{i}",
                                  name=f"dacc_{qc}_{hp}_{i}", bufs=1)
                        for i in range(2)]
                for kt in range(nkt):
                    # stage-ordered so the DVE queue never interleaves a
                    # mask add behind an exp-dependent accumulator add.
                    # Diagonal tile j: cols < j*128 are fully masked, so
                    # scores/exp/mask/acc all narrow to [lo:512] and the PV
                    # matmul reads a pre-zeroed strip.
                    j = kt - qc * 4
                    lo = j * 128 if j >= 1 else 0
                    ps_l, e_l = [], []
                    for i in range(2):
                        h = 2 * hp + i
                        ps_s = psS.tile([128, 512], f32, tag="s",
                                        name=f"s_{qc}_{hp}_{kt}_{i}")
                        nc.tensor.matmul(
                            ps_s[:, lo:512],
                            qk_bf[HL + h][:, kt * 128:(kt + 1) * 128],
                            qk_bf[h][:, qc * 512 + lo:(qc + 1) * 512],
                            start=True, stop=True)
                        ps_l.append(ps_s)
                    if j >= 0:
                        for i in range(2):
                            nc.vector.tensor_tensor(
                                out=ps_l[i][:, j * 128:(j + 1) * 128],
                                in0=ps_l[i][:, j * 128:(j + 1) * 128],
                                in1=mtri[:], op=OP.add)
                    for i in range(2):
                        if j >= 1:
                            e_sb = e_diag[(j, i)]
                        else:
                            e_sb = attp.tile([128, 512], bf16, tag="e",
                                             name=f"e_{qc}_{hp}_{kt}_{i}",
                                             bufs=4)
                        nc.scalar.activation(e_sb[:, lo:512],
                                             ps_l[i][:, lo:512], AF.Exp,
                                             scale=SM_SCALE)
                        e_l.append(e_sb)
                    for i in range(2):
                        if kt == 0:
                            nc.vector.tensor_copy(dacc[i][:], e_l[i][:])
                        else:
                            nc.vector.tensor_tensor(
                                out=dacc[i][:, lo:512],
                                in0=dacc[i][:, lo:512],
                                in1=e_l[i][:, lo:512], op=OP.add)
                    for i in range(2):
                        h = 2 * hp + i
                        # kt=0 is always full-width (j<=0), so the partial
                        # accumulates land on an initialized full bank
                        nc.tensor.matmul(
                            ps_pv[i][:, lo:512],
                            v_bf[kt][:, h * 128:(h + 1) * 128],
                            e_l[i][:, lo:512],
                            start=(kt == 0), stop=(kt == nkt - 1))
                for i in range(2):
                    nc.gpsimd.partition_all_reduce(dacc[i][:], dacc[i][:],
                                                   128, ReduceOp.add)
                for i in range(2):
                    rb = attp.tile([128, 512], f32, tag="rb",
                                   name=f"rb_{qc}_{hp}_{i}", bufs=1)
                    nc.vector.reciprocal_approx_fast(rb[:], dacc[i][:])
                    # o2p column layout is g-major (g = r*2 + i: dest rank r,
                    # pair member i) so the staging DMA is 3-dim
                    nc.vector.tensor_tensor(
                        out=o2p[:].rearrange("p (r i f) -> p i r f",
                                             i=2, f=128)[:, i],
                        in0=ps_pv[i][:].rearrange("p (r f) -> p r f", f=128),
                        in1=rb[:].rearrange("p (r f) -> p r f", f=128),
                        op=OP.mult)
                # stage O^T blocks to DRAM grouped by destination core
                # (same block for both batch-groups' slots) and exchange
                a_in = dram.tile([2048, 128], bf16, tag=f"ain_{qc}_{hp}",
                                 name=f"ain_{qc}_{hp}")
                src = o2p[:].rearrange("p (g f) -> p g f", f=128)
                # only the half read by this core's batch group needs real
                # data; the other half is junk the receivers never read
                for half, cond in ((0, is_b0), (1, is_b1)):
                    dst = a_in[half * 1024:(half + 1) * 1024, :].rearrange(
                        "(g p) f -> p g f", p=128)
                    nc.sync.dma_start(out=dst, in_=src, cond=cond)
                a_out = dram.tile([2048, 128], bf16, tag=f"aout_{qc}_{hp}",
                                  name=f"aout_{qc}_{hp}")
                nc.gpsimd.collective_compute(
                    "AllToAll", OP.bypass, replica_groups=GROUPS8,
                    ins=[a_in[:].opt()], outs=[a_out[:].opt()])
                a2a_out[(qc, hp)] = a_out

            # ---------------- Phase A: QKV projection + RoPE ----------------
            # (+ chunk 0's attention as soon as s-chunk 0 is complete)
            with tc.tile_pool(name="pha", bufs=1) as pha, \
                 tc.tile_pool(name="rope", bufs=1) as ropep, \
                 tc.tile_pool(name="psA", bufs=2, space="PSUM") as psA:
                wq_sb = [pha.tile([128, NDT * 128], bf16, tag=f"wq{et}",
                                  name=f"wq{et}") for et in range(2 * HL)]
                wv_sb = [pha.tile([128, HL * HD], bf16, tag=f"wv{dt}",
                                  name=f"wv{dt}") for dt in range(NDT)]
                xt_t = {}
                tabs = {}

                def prefetch_xt(sc):
                    for dt in range(NDT):
                        t = pha.tile([128, 512], bf16, tag=f"xt{dt}",
                                     name=f"xt_{dt}_{sc}", bufs=2)
                        nc.sync.dma_start(
                            out=t[:],
                            in_=xt_ext[dt * 128:(dt + 1) * 128,
                                       sc * 512:(sc + 1) * 512])
                        xt_t[(dt, sc)] = t

                def prefetch_tab(sc):
                    cl = slice(sc * 512, (sc + 1) * 512)
                    ct = pha.tile([128, 512], f32, tag="c2",
                                  name=f"c2_{sc}", bufs=2)
                    st_ = pha.tile([128, 512], f32, tag="s2",
                                   name=f"s2_{sc}", bufs=2)
                    nc.sync.dma_start(out=ct[:], in_=c2_ext[:, cl])
                    nc.sync.dma_start(out=st_[:], in_=s2_ext[:, cl])
                    tabs[sc] = (ct, st_)

                # DMA issue order tuned so compute starts after ~2.5 MB and
                # the rotary tables are resident before the first RoPE drain
                nc.sync.dma_start(out=wq_sb[0][:], in_=wqk_ext[0:128, :])
                prefetch_xt(0)
                prefetch_tab(0)
                for et in range(1, 2 * HL):
                    nc.sync.dma_start(out=wq_sb[et][:],
                                      in_=wqk_ext[et * 128:(et + 1) * 128, :])
                for dt in range(NDT):
                    nc.sync.dma_start(out=wv_sb[dt][:],
                                      in_=wv_ext[dt * 128:(dt + 1) * 128, :])
                nc.sync.dma_start(out=mtri[:], in_=maskT_ext[:])
                prefetch_xt(1)
                prefetch_tab(1)

                for sc in range(NSC):
                    ct, st_ = tabs[sc]
                    for et in range(2 * HL):
                        ps = psA.tile([128, 512], f32, tag="psA",
                                      name=f"psA_{sc}_{et}")
                        for dt in range(NDT):
                            nc.tensor.matmul(
                                ps[:], wq_sb[et][:, dt * 128:(dt + 1) * 128],
                                xt_t[(dt, sc)][:],
                                start=(dt == 0), stop=(dt == NDT - 1))
                        # u = [r*c; i*c]; w = [-i*s; r*s] (s2 = [-sin; sin],
                        # cross-partition reads stay on the PSUM operand);
                        # qk = u + w = [r*c - i*s; i*c + r*s]
                        cl = slice(sc * 512, (sc + 1) * 512)
                        u = ropep.tile([128, 512], f32, tag="t1",
                                       name=f"t1_{sc}_{et}")
                        w = ropep.tile([128, 512], f32, tag="t2",
                                       name=f"t2_{sc}_{et}")
                        nc.vector.tensor_tensor(out=u[:], in0=ps[:],
                                                in1=ct[:], op=OP.mult)
                        nc.vector.tensor_tensor(out=w[0:64, :],
                                                in0=ps[64:128, :],
                                                in1=st_[0:64, :],
                                                op=OP.mult)
                        nc.vector.tensor_tensor(out=w[64:128, :],
                                                in0=ps[0:64, :],
                                                in1=st_[64:128, :],
                                                op=OP.mult)
                        nc.vector.tensor_tensor(out=qk_bf[et][:, cl],
                                                in0=u[:], in1=w[:], op=OP.add)
                    for stl in range(4):
                        st = sc * 4 + stl
                        psv = psA.tile([128, 512], f32, tag="psA",
                                       name=f"psV_{st}")
                        for dt in range(NDT):
                            nc.tensor.matmul(
                                psv[:],
                                xt_t[(dt, sc)][:, stl * 128:(stl + 1) * 128],
                                wv_sb[dt][:],
                                start=(dt == 0), stop=(dt == NDT - 1))
                        nc.scalar.copy(v_bf[st][:], psv[:])
                    if sc + 2 < NSC:
                        prefetch_xt(sc + 2)
                        prefetch_tab(sc + 2)
                    if sc == 0:
                        attention(0, 0)
                        attention(0, 1)

            # -------- Phase B: remaining attention + A2A + out-proj --------
            with tc.tile_pool(name="phb", bufs=1) as phb, \
                 tc.tile_pool(name="psPR", bufs=2, space="PSUM") as psPR:
                woT_sb = [phb.tile([128, D], bf16, tag=f"wo{k}", name=f"wo{k}")
                          for k in range(NDT)]
                for k in range(NDT):
                    nc.sync.dma_start(out=woT_sb[k][:],
                                      in_=woT_ext[k * 128:(k + 1) * 128, :])

                def load_lhs_hp(qc, hp):
                    # lhs_hp block g = r*2 + i holds global ocol block (head)
                    # k = r*4 + 2*hp + i.  Separate tiles per hp so the
                    # conservative whole-tile deps of the conditional DMAs
                    # don't make the hp0 matmuls wait on the hp1 AllToAll.
                    t = attp.tile([128, 1024], bf16, tag=f"lhs{hp}",
                                  name=f"lhs_{qc}_{hp}", bufs=2)
                    a_out = a2a_out[(qc, hp)]
                    dst = t[:].rearrange("p (g f) -> p g f", f=128)
                    for b, cond in ((0, is_b0), (1, is_b1)):
                        src = a_out[b * 1024:(b + 1) * 1024, :].rearrange(
                            "(g p) f -> p g f", p=128)
                        nc.sync.dma_start(out=dst, in_=src, cond=cond)
                    return t

                def shield(t, tag, bufs=1):
                    # re-copy on the DVE so later consumers depend on the
                    # copy, not on the dynamic-DMA/collective semaphore
                    # chain (which conservatively waits for ALL later
                    # collectives)
                    c = attp.tile([128, 1024], bf16, tag=tag,
                                  name=f"sh_{tag}", bufs=bufs)
                    nc.vector.tensor_copy(c[:], t[:])
                    return c

                def outproj(qc, lhs=None):
                    if lhs is None:
                        lhs = [None, None]
                    lhs = [lhs[hp] if lhs[hp] is not None
                           else load_lhs_hp(qc, hp) for hp in range(2)]
                    # ec columns run in pairs, hp0 blocks first across the
                    # pair, so the first half of the contraction can run
                    # while the hp1 AllToAll is still in flight
                    for eca, ecb in ((0, 1), (2, 3)):
                        pss = {ec: psPR.tile([128, 512], f32, tag="pr",
                                             name=f"pr_{qc}_{ec}")
                               for ec in (eca, ecb)}
                        for hp in range(2):
                            for ec in (eca, ecb):
                                for n in range(8):
                                    r, i = divmod(n, 2)
                                    k = r * HL + 2 * hp + i
                                    nc.tensor.matmul(
                                        pss[ec][:],
                                        lhs[hp][:, n * 128:(n + 1) * 128],
                                        woT_sb[k][:, ec * 512:(ec + 1) * 512],
                                        start=(hp == 0 and n == 0),
                                        stop=(hp == 1 and n == 7))
                        for ec in (eca, ecb):
                            fin = attp.tile([128, 512], bf16, tag="fin",
                                            name=f"fin_{qc}_{ec}", bufs=2)
                            nc.scalar.copy(fin[:], pss[ec][:])
                            nc.sync.dma_start(
                                out=out_ext[qc * 128:(qc + 1) * 128,
                                            ec * 512:(ec + 1) * 512],
                                in_=fin[:])

                # the out-projections of chunks 0 and 3 are deferred (with
                # shielded lhs tiles) to the very end, so ~45us of matmuls
                # covers the final AllToAll; only the last chunk's hp1 half
                # remains exposed
                q0, qa, qb, qlast = CHUNK_ORDER
                attention(qa, 0)
                raw0 = [load_lhs_hp(q0, hp) for hp in range(2)]
                attention(qa, 1)
                attention(qb, 0)
                outproj(qa)
                attention(qb, 1)
                lhs0 = [shield(raw0[hp], f"lhsc{hp}") for hp in range(2)]
                lhsb = [shield(load_lhs_hp(qb, hp), f"lhs{hp}", bufs=2)
                        for hp in range(2)]
                attention(qlast, 0)
                lhs_last_h0 = shield(load_lhs_hp(qlast, 0), "lhsd0")
                attention(qlast, 1)
                outproj(qb, lhs=lhsb)
                outproj(q0, lhs=lhs0)
                outproj(qlast, lhs=[lhs_last_h0, None])
    nc.finalize()
    return nc


def _prep_inputs(x, freqs_cos, freqs_sin, mask, wqkv, wo):
    bf = ml_dtypes.bfloat16
    perm = np.concatenate([np.arange(0, HD, 2), np.arange(1, HD, 2)])
    mask2d = np.asarray(mask, np.float32).reshape(S, S)
    maskT = np.ascontiguousarray(
        np.maximum(mask2d[0:128, 0:128].T, -1e30)).astype(ml_dtypes.bfloat16)
    cosT = np.asarray(freqs_cos, np.float32).T   # [64, S]
    sinT = np.asarray(freqs_sin, np.float32).T
    c2 = np.ascontiguousarray(np.concatenate([cosT, cosT], axis=0))
    s2 = np.ascontiguousarray(np.concatenate([-sinT, sinT], axis=0))
    wqkv = np.asarray(wqkv, np.float32)
    wo = np.asarray(wo, np.float32)
    x = np.asarray(x, np.float32)
    woT = np.ascontiguousarray(wo.T).astype(bf)   # [2048 o, 2048 e]

    in_maps = []
    for c in range(8):
        b, r = divmod(c, TP)
        heads = list(range(r * HL, (r + 1) * HL))
        # q/k weights: per (sec, head) block in SBUF layout [128 p=d%128,
        # (dt c)=hd], i.e. transpose of blk[c, dt*128+p]
        rows = []
        for sec in range(2):
            for h in heads:
                blk = wqkv[sec * D + h * HD: sec * D + (h + 1) * HD][perm]
                b3 = blk.reshape(HD, NDT, 128)          # [hd, dt, p]
                rows.append(np.transpose(b3, (2, 1, 0)).reshape(128, -1))
        wqk = np.ascontiguousarray(np.concatenate(rows, axis=0)).astype(bf)
        wv = np.ascontiguousarray(np.concatenate(
            [wqkv[2 * D + h * HD: 2 * D + (h + 1) * HD].T for h in heads],
            axis=1)).astype(bf)                          # [2048, 512]
        xt = np.ascontiguousarray(x[b].T).astype(bf)
        in_maps.append({"xt": xt, "wqk": wqk, "wv": wv, "c2": c2, "s2": s2,
                        "maskT": maskT, "woT": woT})
    return in_maps


def kernel(x, freqs_cos, freqs_sin, mask, wqkv, wo, input_pos=None,
           _want_res=False, _trace=False, _tmpdir=None):
    from concourse.bass_utils import run_bass_kernel_spmd

    if "nc" not in _cache:
        _cache["nc"] = _build_graph()
    nc = _cache["nc"]

    in_maps = _prep_inputs(x, freqs_cos, freqs_sin, mask, wqkv, wo)
    kw = {}
    if _trace:
        kw = dict(trace=True, tmpdir=_tmpdir)
    res = run_bass_kernel_spmd(nc, in_maps, list(range(8)), **kw)

    y = np.empty((B, S, D), np.float32)
    for c in range(8):
        b, r = divmod(c, TP)
        oc = np.asarray(res.results[c]["out"], np.float32)
        for qc in range(NQC):
            qt = 4 * qc + r
            y[b, qt * 128:(qt + 1) * 128, :] = oc[qc * 128:(qc + 1) * 128]
    if _want_res:
        return y, res
    return y
